# revision 12
# baseline (speedup 1.0000x reference)
"""Trainium2 Bass kernel for nn_CrossAttention_47502338294587.

Math: the reference cross-attention has a single KV position broadcast over
all T query positions.  Softmax over a row of identical logits is uniform,
so attention output == v for every query, and the whole module collapses to

    out[b, t, :] = (visual_features[b] @ Wv + bv) @ Wp + bp      (for all t)

independent of x / Wq / Wk.  The device computes the two projections and
broadcasts the per-batch row over the T axis; the host only does input
layout prep (incl. bf16 weight packing) and shard re-assembly.

Sharding: tensor-parallel over the output channel dim C - core i computes
and writes out[:, :, i*128:(i+1)*128] (full Wv, column shard of Wp / bp).

v4: the two HWDGE queues carry only the wv chunk stream (+ tiny vfti);
wp and a packed consts tensor (sel / bp row / ones / bv) ride the idle
gpsimd SWDGE queue.  PE kept warm with early K=1 dummy matmuls (HAM clock
gate).  bv enters the mm1 PSUM group as the leading K=1 start=True matmul
so the group completes right at the last wv chunk.  bp sits pre-placed in
row 4 of the sel region, which the DVE rhs5 multiply updates in place, so
the broadcast matmul (bf16, K=5) reads sel/bp directly.  All PSUM->SBUF
copies on vector (no ACT tables on the scalar engine).  Output: four
replicated-source DMAs (t-chunk pairs) with 2KB descriptors.

Per-core structure:
  mm1:   psum_vv[h] = bv (K=1, start) + sum_k vfT_k^T @ Wv_k[:, h]  (bf16)
  tr:    vv -> vv^T chunks via PE transpose (bf16)
  mm2:   prow = sum_k vvT_k^T @ Wp_k          [4, 128] f32 psum
  bcast: sel[0:4] *= rep4(prow) in place (DVE), sel row 4 = bp (host)
         pbc[t, (b,c)] = ones5^T @ sel5  (one bf16 matmul, K=5)
         four replicated-source DMAs (t-chunk pairs) write the shard
"""

import os
import sys

import numpy as np

for _p in ("/opt/trn_rl_repo",):
    if _p not in sys.path and os.path.isdir(_p):
        sys.path.insert(0, _p)

B, T, C = 4, 1024, 1024
N_CORES = 8
CSH = C // N_CORES  # 128, C-shard per core
KC = C // 128  # 8 contraction chunks
N_WARM = 8  # PE warmup dummy matmuls (HAM clock gate)

_BUILT = None


def build_nc():
    """Build + compile the Bass program (one NeuronCore's SPMD body)."""
    import concourse.bass as bass
    import concourse.mybir as mybir
    import concourse.tile as tile
    from concourse import bacc
    from concourse.bass import ts

    f32 = mybir.dt.float32
    bf16 = mybir.dt.bfloat16
    nc = bacc.Bacc("TRN2", target_bir_lowering=False, debug=False)

    # ---- DRAM inputs (host pre-packed layouts) --------------------------
    # wv pair j: [p, r*1024 + n] = bf16(Wv[(2j+r)*128 + p, n])  (4KB desc rows)
    wv_d = [
        nc.dram_tensor(f"wv{j}", [128, 2 * C], bf16, kind="ExternalInput")
        for j in range(KC // 2)
    ]
    # vfti[p, 0:32] = vfT chunks: [p, k*4 + b] = vf[b, k*128 + p]
    # vfti[0:4, 32:36] = eye(4); vfti[0:1, 36:40] = ones (K=1 bias lhsT)
    vfti_d = nc.dram_tensor("vfti", [128, 40], bf16, kind="ExternalInput")
    # wp_p[p, k*CSH + c] = bf16(Wp[k*128 + p, ci_c])
    wp_d = nc.dram_tensor("wp_p", [128, KC * CSH], bf16, kind="ExternalInput")
    # consts pack [5, 1664]:
    #   [0:4, 0:512]   sel ((k==b) block mask; becomes rhs5 rows 0-3 in place)
    #   [4:5, 0:512]   bp row (tiled 4x) = rhs5 row 4
    #   [0:5, 512:640] ones5
    #   [0:1, 640:1664] bv row
    consts_d = nc.dram_tensor("consts5", [5, 1664], bf16, kind="ExternalInput")
    # out[t, b, c_local]; host re-assembles full[b, t, ci] = out[t, b, :]
    out = nc.dram_tensor("out", [T, B, CSH], bf16, kind="ExternalOutput")

    with tile.TileContext(nc) as tc:
        with (
            tc.tile_pool(name="sb", bufs=1) as sb,
            tc.tile_pool(name="pv", bufs=1, space="PSUM") as pv,
            tc.tile_pool(name="pt", bufs=1, space="PSUM") as pt,
            tc.tile_pool(name="pr", bufs=1, space="PSUM") as pr,
            tc.tile_pool(name="pb", bufs=1, space="PSUM") as pb,
        ):
            # ---- SBUF tiles -------------------------------------------------
            wv_t = [
                sb.tile([128, 2, C], bf16, name=f"wv{j}", tag=f"wv{j}")
                for j in range(KC // 2)
            ]
            vfti_t = sb.tile([128, 40], bf16, tag="vfti")
            wp_t = sb.tile([128, KC, CSH], bf16, tag="wp_t")
            consts_t = sb.tile([5, 1664], bf16, tag="consts5")
            vv_sb = [
                sb.tile([B, 512], bf16, name=f"vv{h}", tag=f"vv{h}")
                for h in range(2)
            ]
            vvt_sb = [
                sb.tile([128, 4, B], bf16, name=f"vvt{h}", tag=f"vvt{h}")
                for h in range(2)
            ]
            bc_t = sb.tile([128, B * CSH], bf16, tag="bc")
            warm_t = sb.tile([1, 640], bf16, tag="warm")

            vft = vfti_t[:, 0:32].rearrange("p (k b) -> p k b", b=B)
            ident = vfti_t[0:4, 32:36]
            ones1 = vfti_t[0:1, 36:40]
            sel5 = consts_t[0:5, 0:512]
            sel4 = consts_t[0:4, 0:512]
            ones5 = consts_t[0:5, 512:640]
            bv_row = consts_t[0:1, 640:1664]

            # ---- PSUM tiles -------------------------------------------------
            psum_vv = [
                pv.tile([B, 512], f32, name=f"pvv{h}", tag=f"pvv{h}")
                for h in range(2)
            ]
            psum_vvt = [
                pt.tile([128, 4, B], bf16, name=f"pvt{h}", tag=f"pvt{h}")
                for h in range(2)
            ]
            psum_row = pr.tile([B, CSH], f32, tag="pr")
            psum_bc = pb.tile([128, B * CSH], f32, tag="pb")

            # ---- DMA in -----------------------------------------------------
            # sync HWDGE queue: consts, wp, wv pairs 0/2 (+ out q0-3)
            # scalar HWDGE queue: vfti, wv pairs 1/3 (+ out q4-7)
            nc.sync.dma_start(consts_t[:], consts_d[:, :])
            nc.scalar.dma_start(vfti_t[:], vfti_d[:, :])
            nc.sync.dma_start(
                wp_t[:], wp_d.rearrange("p (k c) -> p k c", c=CSH)
            )
            for j in range(KC // 2):
                eng = nc.sync if j % 2 == 0 else nc.scalar
                eng.dma_start(
                    wv_t[j][:], wv_d[j].rearrange("p (r n) -> p r n", r=2)
                )

            # ---- PE warmup: K=1 dummies into psum_bc (overwritten later) ----
            # memset on the otherwise-idle gpsimd so dummies start asap
            nc.gpsimd.memset(warm_t[:], 1.0)
            for w in range(N_WARM):
                nc.tensor.matmul(
                    psum_bc[:, 0:512],
                    warm_t[0:1, 0:128],
                    warm_t[0:1, 128:640],
                    start=True,
                    stop=True,
                )

            # ---- mm1: psum_vv[h] = bv + sum_k vfT_k^T @ Wv_k[:, h] ----------
            # leading K=1 bias row opens the accumulation group
            for h in range(2):
                nc.tensor.matmul(
                    psum_vv[h][:],
                    ones1,
                    bv_row[:, ts(h, 512)],
                    start=True,
                    stop=False,
                )
            for k in range(KC):
                j, r = k // 2, k % 2
                for h in range(2):
                    nc.tensor.matmul(
                        psum_vv[h][:],
                        vft[:, k, :],
                        wv_t[j][:, r, ts(h, 512)],
                        start=False,
                        stop=(k == KC - 1),
                    )

            # ---- transpose vv -> vv^T chunks, then mm2 ----------------------
            nc.vector.tensor_copy(vv_sb[0][:], psum_vv[0][:])
            nc.vector.tensor_copy(vv_sb[1][:], psum_vv[1][:])

            for h in range(2):
                for j in range(4):
                    nc.tensor.transpose(
                        psum_vvt[h][:, j, :],
                        vv_sb[h][0:B, ts(j, 128)],
                        ident,
                    )
            nc.vector.tensor_copy(vvt_sb[0][:], psum_vvt[0][:])
            nc.vector.tensor_copy(vvt_sb[1][:], psum_vvt[1][:])

            # mm2: prow = sum_k vvT_k^T @ Wp_k   [4, 128] f32
            for k in range(KC):
                nc.tensor.matmul(
                    psum_row[:],
                    vvt_sb[k // 4][:, k % 4, :],
                    wp_t[:, k, :],
                    start=(k == 0),
                    stop=(k == KC - 1),
                )

            # sel rows 0-3 *= rep4(prow) in place (row 4 = bp, host-placed)
            pra = psum_row[:]
            prep = bass.AP(
                pra.tensor,
                pra.offset,
                [list(pra.ap[0]), [0, B], list(pra.ap[1])],
            )
            sel4v = sel4.rearrange("p (q f) -> p q f", q=B)
            nc.vector.tensor_mul(sel4v, prep, sel4v)
            # bcast: pbc[t, (b,c)] = ones5^T @ sel5   (K=5, bf16)
            nc.tensor.matmul(
                psum_bc[:],
                ones5,
                sel5,
                start=True,
                stop=True,
            )
            nc.vector.tensor_copy(bc_t[:], psum_bc[:])

            # out DMAs: replicated source over t-chunk pairs; 2KB descs.
            out_v = out.rearrange("(q p) b c -> p q (b c)", p=128)
            bca = bc_t[:]
            rep = bass.AP(
                bca.tensor,
                bca.offset,
                [list(bca.ap[0]), [0, KC // 2], list(bca.ap[1])],
            )
            nc.sync.dma_start(out_v[:, 0 : KC // 2, :], rep)
            nc.scalar.dma_start(out_v[:, KC // 2 : KC, :], rep)

    nc.compile()
    return nc


def _get_built():
    global _BUILT
    if _BUILT is None:
        _BUILT = build_nc()
    return _BUILT


def make_in_maps(inputs):
    import ml_dtypes

    bf16 = ml_dtypes.bfloat16

    vf = np.asarray(inputs["visual_features"], np.float32)
    wv = np.asarray(inputs["Wv"], np.float32)
    wp = np.asarray(inputs["Wp"], np.float32)
    bv = np.asarray(inputs["bv"], np.float32)
    bp = np.asarray(inputs["bp"], np.float32)

    wv_bf = wv.astype(bf16)
    # pair j: [p, r*1024 + n] = Wv[(2j+r)*128 + p, n]
    wv_chunks = [
        np.ascontiguousarray(
            wv_bf[2 * j * 128 : (2 * j + 2) * 128, :]
            .reshape(2, 128, C)
            .transpose(1, 0, 2)
            .reshape(128, 2 * C)
        )
        for j in range(KC // 2)
    ]

    # vfti pack: vfT chunks + eye(4) + ones row
    vfti = np.zeros((128, 40), np.float32)
    vfti[:, 0:32] = vf.T.reshape(KC, 128, B).transpose(1, 0, 2).reshape(128, KC * B)
    vfti[0:4, 32:36] = np.eye(4, dtype=np.float32)
    vfti[0:1, 36:40] = 1.0
    vfti = vfti.astype(bf16)

    # consts pack: sel + bp row + ones5 + bv row (bp per-core, rest shared)
    consts_base = np.zeros((5, 1664), np.float32)
    for b in range(B):
        consts_base[b, b * CSH : (b + 1) * CSH] = 1.0
    consts_base[:, 512:640] = 1.0
    consts_base[0, 640:1664] = bv

    maps = []
    for i in range(N_CORES):
        ci = slice(i * CSH, (i + 1) * CSH)
        # wp_p[p, k*CSH + c] = Wp[k*128 + p, ci_c]
        wp_p = np.ascontiguousarray(
            wp[:, ci].reshape(KC, 128, CSH).transpose(1, 0, 2).reshape(128, KC * CSH)
        ).astype(bf16)
        consts5 = consts_base.copy()
        consts5[4, 0:512] = np.tile(bp[ci], B)
        m = {
            "vfti": vfti,
            "wp_p": wp_p,
            "consts5": consts5.astype(bf16),
        }
        for j in range(KC // 2):
            m[f"wv{j}"] = wv_chunks[j]
        maps.append(m)
    return maps


def run(inputs, trace=False, **kw):
    from concourse.bass_utils import run_bass_kernel_spmd

    nc = _get_built()
    res = run_bass_kernel_spmd(
        nc,
        make_in_maps(inputs),
        core_ids=list(range(N_CORES)),
        trace=trace,
        **kw,
    )
    full = np.empty((B, T, C), np.float32)
    for i, r in enumerate(res.results):
        full[:, :, i * CSH : (i + 1) * CSH] = r["out"].transpose(1, 0, 2).astype(np.float32)
    return full, res


def kernel(**inputs) -> np.ndarray:
    full, _ = run(inputs, trace=False)
    return full


# revision 25
# speedup vs baseline: 1.1510x; 1.1510x over previous
"""Trainium2 Bass kernel for nn_CrossAttention_47502338294587.

Math: the reference cross-attention has a single KV position broadcast over
all T query positions.  Softmax over a row of identical logits is uniform,
so attention output == v for every query, and the whole module collapses to

    out[b, t, :] = (visual_features[b] @ Wv + bv) @ Wp + bp      (for all t)

independent of x / Wq / Wk.  The device computes the two projections and
broadcasts the per-batch row over the T axis; the host only does input
layout prep (incl. bf16 weight packing) and shard re-assembly.

Sharding: tensor-parallel over the output channel dim C - core i computes
and writes out[:, :, i*128:(i+1)*128] (full Wv, column shard of Wp / bp).

All tensor-engine math runs in bf16 (f32 PSUM accumulation); the wv
stream is split into 8 x 256KB chunk DMAs alternating across the two HWDGE
queues so mm1 pipelines behind the DMA stream.  bv enters the mm1 PSUM
group as the leading K=1 start=True matmul.  bp sits pre-placed in row 4
of the sel region, which the DVE rhs5 multiply updates in place, so the
broadcast matmul (bf16, K=5) reads sel/bp directly.  All PSUM->SBUF copies
on vector (no ACT tables on the scalar engine).  The output is written
bf16 in a p-major layout ([p, q, b, c] -> 8KB contiguous per partition,
one DMA); the host transposes and upcasts to f32 during re-assembly.

Per-core structure:
  mm1:   psum_vv[h] = bv (K=1, start) + sum_k vfT_k^T @ Wv_k[:, h]  (bf16)
  tr:    vv -> vv^T chunks via PE transpose (bf16)
  mm2:   prow = sum_k vvT_k^T @ Wp_k          [4, 128] f32 psum
  bcast: sel[0:4] *= rep4(prow) in place (DVE), sel row 4 = bp (host)
         pbc[t, (b,c)] = ones5^T @ sel5  (one bf16 matmul, K=5)
         one replicated-source p-major DMA writes the whole shard
"""

import os
import sys

import numpy as np

for _p in ("/opt/trn_rl_repo",):
    if _p not in sys.path and os.path.isdir(_p):
        sys.path.insert(0, _p)

B, T, C = 4, 1024, 1024
N_CORES = 8
CSH = C // N_CORES  # 128, C-shard per core
KC = C // 128  # 8 contraction chunks

_BUILT = None


def build_nc():
    """Build + compile the Bass program (one NeuronCore's SPMD body)."""
    import concourse.bass as bass
    import concourse.mybir as mybir
    import concourse.tile as tile
    from concourse import bacc
    from concourse.bass import ts

    f32 = mybir.dt.float32
    bf16 = mybir.dt.bfloat16
    nc = bacc.Bacc("TRN2", target_bir_lowering=False, debug=False)

    # ---- DRAM inputs (host pre-packed layouts) --------------------------
    # wv_k[p, n] = bf16(Wv[k*128 + p, n])
    wv_d = [
        nc.dram_tensor(f"wv{k}", [128, C], bf16, kind="ExternalInput")
        for k in range(KC)
    ]
    # vfti[p, 0:32] = vfT chunks: [p, k*4 + b] = vf[b, k*128 + p]
    # vfti[0:4, 32:36] = eye(4); vfti[0:1, 36:40] = ones (K=1 bias lhsT)
    vfti_d = nc.dram_tensor("vfti", [128, 40], bf16, kind="ExternalInput")
    # wp_p[p, k*CSH + c] = bf16(Wp[k*128 + p, ci_c])
    wp_d = nc.dram_tensor("wp_p", [128, KC * CSH], bf16, kind="ExternalInput")
    # consts pack [5, 1664]:
    #   [0:4, 0:512]   sel ((k==b) block mask; becomes rhs5 rows 0-3 in place)
    #   [4:5, 0:512]   bp row (tiled 4x) = rhs5 row 4
    #   [0:5, 512:640] ones5
    #   [0:1, 640:1664] bv row
    consts_d = nc.dram_tensor("consts5", [5, 1664], bf16, kind="ExternalInput")
    # out[p, q, b, c_local] (p-major: 8KB contiguous per partition);
    # host re-assembles full[b, q*128 + p, ci] = out[p, q, b, :]
    out = nc.dram_tensor("out", [128, KC, B, CSH], bf16, kind="ExternalOutput")

    with tile.TileContext(nc) as tc:
        with (
            tc.tile_pool(name="sb", bufs=1) as sb,
            tc.tile_pool(name="pv", bufs=1, space="PSUM") as pv,
            tc.tile_pool(name="pt", bufs=1, space="PSUM") as pt,
            tc.tile_pool(name="pr", bufs=1, space="PSUM") as pr,
            tc.tile_pool(name="pb", bufs=1, space="PSUM") as pb,
        ):
            # ---- SBUF tiles -------------------------------------------------
            wv_t = [
                sb.tile([128, C], bf16, name=f"wv{k}", tag=f"wv{k}")
                for k in range(KC)
            ]
            vfti_t = sb.tile([128, 40], bf16, tag="vfti")
            wp_t = sb.tile([128, KC, CSH], bf16, tag="wp_t")
            consts_t = sb.tile([5, 1664], bf16, tag="consts5")
            vv_sb = [
                sb.tile([B, 512], bf16, name=f"vv{h}", tag=f"vv{h}")
                for h in range(2)
            ]
            vvt_sb = [
                sb.tile([128, 4, B], bf16, name=f"vvt{h}", tag=f"vvt{h}")
                for h in range(2)
            ]
            bc_t = sb.tile([128, B * CSH], bf16, tag="bc")

            vft = vfti_t[:, 0:32].rearrange("p (k b) -> p k b", b=B)
            ident = vfti_t[0:4, 32:36]
            ones1 = vfti_t[0:1, 36:40]
            sel5 = consts_t[0:5, 0:512]
            sel4 = consts_t[0:4, 0:512]
            ones5 = consts_t[0:5, 512:640]
            bv_row = consts_t[0:1, 640:1664]

            # ---- PSUM tiles -------------------------------------------------
            psum_vv = [
                pv.tile([B, 512], f32, name=f"pvv{h}", tag=f"pvv{h}")
                for h in range(2)
            ]
            psum_vvt = [
                pt.tile([128, 4, B], bf16, name=f"pvt{h}", tag=f"pvt{h}")
                for h in range(2)
            ]
            psum_row = pr.tile([B, CSH], f32, tag="pr")
            psum_bc = pb.tile([128, B * CSH], f32, tag="pb")

            # ---- DMA in -----------------------------------------------------
            # sync HWDGE queue: consts, wv 0/2/4/6, wp (+ out)
            # scalar HWDGE queue: vfti, wv 1/3/5/7
            nc.sync.dma_start(consts_t[:], consts_d[:, :])
            nc.scalar.dma_start(vfti_t[:], vfti_d[:, :])
            for k in range(KC):
                eng = nc.sync if k % 2 == 0 else nc.scalar
                eng.dma_start(wv_t[k][:], wv_d[k][:, :])
            nc.sync.dma_start(
                wp_t[:], wp_d.rearrange("p (k c) -> p k c", c=CSH)
            )

            # ---- mm1: psum_vv[h] = bv + sum_k vfT_k^T @ Wv_k[:, h] ----------
            # leading K=1 bias row opens the accumulation group
            for h in range(2):
                nc.tensor.matmul(
                    psum_vv[h][:],
                    ones1,
                    bv_row[:, ts(h, 512)],
                    start=True,
                    stop=False,
                )
            for k in range(KC):
                for h in range(2):
                    nc.tensor.matmul(
                        psum_vv[h][:],
                        vft[:, k, :],
                        wv_t[k][:, ts(h, 512)],
                        start=False,
                        stop=(k == KC - 1),
                    )

            # ---- transpose vv -> vv^T chunks, then mm2 ----------------------
            nc.vector.tensor_copy(vv_sb[0][:], psum_vv[0][:])
            nc.vector.tensor_copy(vv_sb[1][:], psum_vv[1][:])

            for h in range(2):
                for j in range(4):
                    nc.tensor.transpose(
                        psum_vvt[h][:, j, :],
                        vv_sb[h][0:B, ts(j, 128)],
                        ident,
                    )
            nc.vector.tensor_copy(vvt_sb[0][:], psum_vvt[0][:])
            nc.vector.tensor_copy(vvt_sb[1][:], psum_vvt[1][:])

            # mm2: prow = sum_k vvT_k^T @ Wp_k   [4, 128] f32
            for k in range(KC):
                nc.tensor.matmul(
                    psum_row[:],
                    vvt_sb[k // 4][:, k % 4, :],
                    wp_t[:, k, :],
                    start=(k == 0),
                    stop=(k == KC - 1),
                )

            # sel rows 0-3 *= rep4(prow) in place (row 4 = bp, host-placed)
            pra = psum_row[:]
            prep = bass.AP(
                pra.tensor,
                pra.offset,
                [list(pra.ap[0]), [0, B], list(pra.ap[1])],
            )
            sel4v = sel4.rearrange("p (q f) -> p q f", q=B)
            nc.vector.tensor_mul(sel4v, prep, sel4v)
            # bcast: pbc[t, (b,c)] = ones5^T @ sel5   (K=5, bf16)
            nc.tensor.matmul(
                psum_bc[:],
                ones5,
                sel5,
                start=True,
                stop=True,
            )
            nc.vector.tensor_copy(bc_t[:], psum_bc[:])

            # out DMA: p-major dest => one 8KB contiguous run per partition
            out_v = out.rearrange("p q b c -> p q (b c)")
            bca = bc_t[:]
            rep = bass.AP(
                bca.tensor,
                bca.offset,
                [list(bca.ap[0]), [0, KC], list(bca.ap[1])],
            )
            nc.sync.dma_start(out_v[:, :, :], rep)

    nc.compile()
    return nc


def _get_built():
    global _BUILT
    if _BUILT is None:
        _BUILT = build_nc()
    return _BUILT


def make_in_maps(inputs):
    import ml_dtypes

    bf16 = ml_dtypes.bfloat16

    vf = np.asarray(inputs["visual_features"], np.float32)
    wv = np.asarray(inputs["Wv"], np.float32)
    wp = np.asarray(inputs["Wp"], np.float32)
    bv = np.asarray(inputs["bv"], np.float32)
    bp = np.asarray(inputs["bp"], np.float32)

    wv_bf = wv.astype(bf16)
    wv_chunks = [
        np.ascontiguousarray(wv_bf[k * 128 : (k + 1) * 128, :]) for k in range(KC)
    ]

    # vfti pack: vfT chunks + eye(4) + ones row
    vfti = np.zeros((128, 40), np.float32)
    vfti[:, 0:32] = vf.T.reshape(KC, 128, B).transpose(1, 0, 2).reshape(128, KC * B)
    vfti[0:4, 32:36] = np.eye(4, dtype=np.float32)
    vfti[0:1, 36:40] = 1.0
    vfti = vfti.astype(bf16)

    # consts pack: sel + bp row + ones5 + bv row (bp per-core, rest shared)
    consts_base = np.zeros((5, 1664), np.float32)
    for b in range(B):
        consts_base[b, b * CSH : (b + 1) * CSH] = 1.0
    consts_base[:, 512:640] = 1.0
    consts_base[0, 640:1664] = bv

    maps = []
    for i in range(N_CORES):
        ci = slice(i * CSH, (i + 1) * CSH)
        # wp_p[p, k*CSH + c] = Wp[k*128 + p, ci_c]
        wp_p = np.ascontiguousarray(
            wp[:, ci].reshape(KC, 128, CSH).transpose(1, 0, 2).reshape(128, KC * CSH)
        ).astype(bf16)
        consts5 = consts_base.copy()
        consts5[4, 0:512] = np.tile(bp[ci], B)
        m = {
            "vfti": vfti,
            "wp_p": wp_p,
            "consts5": consts5.astype(bf16),
        }
        for k in range(KC):
            m[f"wv{k}"] = wv_chunks[k]
        maps.append(m)
    return maps


def run(inputs, trace=False, **kw):
    from concourse.bass_utils import run_bass_kernel_spmd

    nc = _get_built()
    res = run_bass_kernel_spmd(
        nc,
        make_in_maps(inputs),
        core_ids=list(range(N_CORES)),
        trace=trace,
        **kw,
    )
    full = np.empty((B, T, C), np.float32)
    for i, r in enumerate(res.results):
        # out[p, q, b, c] -> full[b, q*128 + p, ci_c]
        o = np.asarray(r["out"]).transpose(2, 1, 0, 3).reshape(B, T, CSH)
        full[:, :, i * CSH : (i + 1) * CSH] = o.astype(np.float32)
    return full, res


def kernel(**inputs) -> np.ndarray:
    full, _ = run(inputs, trace=False)
    return full


# revision 27
# speedup vs baseline: 1.1541x; 1.0027x over previous
"""Trainium2 Bass kernel for nn_CrossAttention_47502338294587.

Math: the reference cross-attention has a single KV position broadcast over
all T query positions.  Softmax over a row of identical logits is uniform,
so attention output == v for every query, and the whole module collapses to

    out[b, t, :] = (visual_features[b] @ Wv + bv) @ Wp + bp      (for all t)

independent of x / Wq / Wk.  The device computes the two projections and
broadcasts the per-batch row over the T axis; the host only does input
layout prep (incl. bf16 weight packing) and shard re-assembly.

Sharding: tensor-parallel over the output channel dim C - core i computes
and writes out[:, :, i*128:(i+1)*128] (full Wv, column shard of Wp / bp).

All tensor-engine math runs in bf16 (f32 PSUM accumulation); the wv
stream is split into 8 x 256KB chunk DMAs alternating across the two HWDGE
queues so mm1 pipelines behind the DMA stream.  bv enters the mm1 PSUM
group as the leading K=1 start=True matmul.  bp sits pre-placed in row 4
of the sel region, which the DVE rhs5 multiply updates in place, so the
broadcast matmul (bf16, K=5) reads sel/bp directly.  All PSUM->SBUF copies
on vector (no ACT tables on the scalar engine).  The output is written
bf16 in a p-major layout ([p, q, b, c] -> 8KB contiguous per partition,
one DMA); the host transposes and upcasts to f32 during re-assembly.

Per-core structure:
  mm1:   psum_vv[h] = bv (K=1, start) + sum_k vfT_k^T @ Wv_k[:, h]  (bf16)
  tr:    vv -> vv^T chunks via PE transpose (bf16)
  mm2:   prow = sum_k vvT_k^T @ Wp_k          [4, 128] f32 psum
  bcast: sel[0:4] *= rep4(prow) in place (DVE), sel row 4 = bp (host)
         pbc[t, (b,c)] = ones5^T @ sel5  (one bf16 matmul, K=5)
         one replicated-source p-major DMA writes the whole shard
"""

import os
import sys

import numpy as np

for _p in ("/opt/trn_rl_repo",):
    if _p not in sys.path and os.path.isdir(_p):
        sys.path.insert(0, _p)

B, T, C = 4, 1024, 1024
N_CORES = 8
CSH = C // N_CORES  # 128, C-shard per core
KC = C // 128  # 8 contraction chunks

_BUILT = None


def build_nc():
    """Build + compile the Bass program (one NeuronCore's SPMD body)."""
    import concourse.bass as bass
    import concourse.mybir as mybir
    import concourse.tile as tile
    from concourse import bacc
    from concourse.bass import ts

    f32 = mybir.dt.float32
    bf16 = mybir.dt.bfloat16
    nc = bacc.Bacc("TRN2", target_bir_lowering=False, debug=False)

    # ---- DRAM inputs (host pre-packed layouts) --------------------------
    # wv_k[p, n] = bf16(Wv[k*128 + p, n])
    wv_d = [
        nc.dram_tensor(f"wv{k}", [128, C], bf16, kind="ExternalInput")
        for k in range(KC)
    ]
    # vfti[p, 0:32] = vfT chunks: [p, k*4 + b] = vf[b, k*128 + p]
    # vfti[0:4, 32:36] = eye(4); vfti[0:1, 36:40] = ones (K=1 bias lhsT)
    vfti_d = nc.dram_tensor("vfti", [128, 40], bf16, kind="ExternalInput")
    # wp_p[p, k*CSH + c] = bf16(Wp[k*128 + p, ci_c])
    wp_d = nc.dram_tensor("wp_p", [128, KC * CSH], bf16, kind="ExternalInput")
    # consts pack [5, 1664]:
    #   [0:4, 0:512]   sel ((k==b) block mask; becomes rhs5 rows 0-3 in place)
    #   [4:5, 0:512]   bp row (tiled 4x) = rhs5 row 4
    #   [0:5, 512:640] ones5
    #   [0:1, 640:1664] bv row
    consts_d = nc.dram_tensor("consts5", [5, 1664], bf16, kind="ExternalInput")
    # out[p, q, b, c_local] (p-major: 8KB contiguous per partition);
    # host re-assembles full[b, q*128 + p, ci] = out[p, q, b, :]
    out = nc.dram_tensor("out", [128, KC, B, CSH], bf16, kind="ExternalOutput")

    with tile.TileContext(nc) as tc:
        with (
            tc.tile_pool(name="sb", bufs=1) as sb,
            tc.tile_pool(name="pv", bufs=1, space="PSUM") as pv,
            tc.tile_pool(name="pt", bufs=1, space="PSUM") as pt,
            tc.tile_pool(name="pr", bufs=1, space="PSUM") as pr,
            tc.tile_pool(name="pb", bufs=1, space="PSUM") as pb,
        ):
            # ---- SBUF tiles -------------------------------------------------
            wv_t = [
                sb.tile([128, C], bf16, name=f"wv{k}", tag=f"wv{k}")
                for k in range(KC)
            ]
            vfti_t = sb.tile([128, 40], bf16, tag="vfti")
            wp_t = sb.tile([128, KC, CSH], bf16, tag="wp_t")
            consts_t = sb.tile([5, 1664], bf16, tag="consts5")
            vv_sb = [
                sb.tile([B, 512], bf16, name=f"vv{h}", tag=f"vv{h}")
                for h in range(2)
            ]
            vvt_sb = [
                sb.tile([128, 4, B], bf16, name=f"vvt{h}", tag=f"vvt{h}")
                for h in range(2)
            ]
            bc_t = sb.tile([128, B * CSH], bf16, tag="bc")

            vft = vfti_t[:, 0:32].rearrange("p (k b) -> p k b", b=B)
            ident = vfti_t[0:4, 32:36]
            ones1 = vfti_t[0:1, 36:40]
            sel5 = consts_t[0:5, 0:512]
            sel4 = consts_t[0:4, 0:512]
            ones5 = consts_t[0:5, 512:640]
            bv_row = consts_t[0:1, 640:1664]

            # ---- PSUM tiles -------------------------------------------------
            psum_vv = [
                pv.tile([B, 512], f32, name=f"pvv{h}", tag=f"pvv{h}")
                for h in range(2)
            ]
            psum_vvt = [
                pt.tile([128, 4, B], bf16, name=f"pvt{h}", tag=f"pvt{h}")
                for h in range(2)
            ]
            psum_row = pr.tile([B, CSH], f32, tag="pr")
            psum_bc = pb.tile([128, B * CSH], f32, tag="pb")

            # ---- DMA in -----------------------------------------------------
            # sync HWDGE queue: consts, wv 0/2/4/6, wp (+ out)
            # scalar HWDGE queue: vfti, wv 1/3/5/7
            nc.sync.dma_start(consts_t[:], consts_d[:, :])
            nc.scalar.dma_start(vfti_t[:], vfti_d[:, :])
            for k in range(KC):
                eng = nc.sync if k % 2 == 0 else nc.scalar
                eng.dma_start(wv_t[k][:], wv_d[k][:, :])
            nc.sync.dma_start(
                wp_t[:], wp_d.rearrange("p (k c) -> p k c", c=CSH)
            )

            # ---- mm1: psum_vv[h] = bv + sum_k vfT_k^T @ Wv_k[:, h] ----------
            # leading K=1 bias row opens the accumulation group
            for h in range(2):
                nc.tensor.matmul(
                    psum_vv[h][:],
                    ones1,
                    bv_row[:, ts(h, 512)],
                    start=True,
                    stop=False,
                )
            for k in range(KC):
                for h in range(2):
                    nc.tensor.matmul(
                        psum_vv[h][:],
                        vft[:, k, :],
                        wv_t[k][:, ts(h, 512)],
                        start=False,
                        stop=(k == KC - 1),
                    )

            # ---- transpose vv -> vv^T chunks, then mm2 ----------------------
            nc.vector.tensor_copy(vv_sb[0][:], psum_vv[0][:])
            nc.vector.tensor_copy(vv_sb[1][:], psum_vv[1][:])

            for h in range(2):
                for j in range(4):
                    nc.tensor.transpose(
                        psum_vvt[h][:, j, :],
                        vv_sb[h][0:B, ts(j, 128)],
                        ident,
                    )
            nc.vector.tensor_copy(vvt_sb[0][:], psum_vvt[0][:])
            nc.vector.tensor_copy(vvt_sb[1][:], psum_vvt[1][:])

            # mm2: prow = sum_k vvT_k^T @ Wp_k   [4, 128] f32
            for k in range(KC):
                nc.tensor.matmul(
                    psum_row[:],
                    vvt_sb[k // 4][:, k % 4, :],
                    wp_t[:, k, :],
                    start=(k == 0),
                    stop=(k == KC - 1),
                )

            # sel rows 0-3 *= rep4(prow) in place (row 4 = bp, host-placed)
            pra = psum_row[:]
            prep = bass.AP(
                pra.tensor,
                pra.offset,
                [list(pra.ap[0]), [0, B], list(pra.ap[1])],
            )
            sel4v = sel4.rearrange("p (q f) -> p q f", q=B)
            nc.vector.tensor_mul(sel4v, prep, sel4v)
            # bcast: pbc[t, (b,c)] = ones5^T @ sel5   (K=5, bf16)
            nc.tensor.matmul(
                psum_bc[:],
                ones5,
                sel5,
                start=True,
                stop=True,
            )
            nc.vector.tensor_copy(bc_t[:], psum_bc[:])

            # out DMA: p-major dest => one 8KB contiguous run per partition
            out_v = out.rearrange("p q b c -> p q (b c)")
            bca = bc_t[:]
            rep = bass.AP(
                bca.tensor,
                bca.offset,
                [list(bca.ap[0]), [0, KC], list(bca.ap[1])],
            )
            nc.sync.dma_start(out_v[:, :, :], rep)

    nc.compile()
    return nc


def _get_built():
    global _BUILT
    if _BUILT is None:
        _BUILT = build_nc()
    return _BUILT


def make_in_maps(inputs):
    import ml_dtypes

    bf16 = ml_dtypes.bfloat16

    vf = np.asarray(inputs["visual_features"], np.float32)
    wv = np.asarray(inputs["Wv"], np.float32)
    wp = np.asarray(inputs["Wp"], np.float32)
    bv = np.asarray(inputs["bv"], np.float32)
    bp = np.asarray(inputs["bp"], np.float32)

    wv_bf = wv.astype(bf16)
    wv_chunks = [
        np.ascontiguousarray(wv_bf[k * 128 : (k + 1) * 128, :]) for k in range(KC)
    ]

    # vfti pack: vfT chunks + eye(4) + ones row
    vfti = np.zeros((128, 40), np.float32)
    vfti[:, 0:32] = vf.T.reshape(KC, 128, B).transpose(1, 0, 2).reshape(128, KC * B)
    vfti[0:4, 32:36] = np.eye(4, dtype=np.float32)
    vfti[0:1, 36:40] = 1.0
    vfti = vfti.astype(bf16)

    # consts pack: sel + bp row + ones5 + bv row (bp per-core, rest shared)
    consts_base = np.zeros((5, 1664), np.float32)
    for b in range(B):
        consts_base[b, b * CSH : (b + 1) * CSH] = 1.0
    consts_base[:, 512:640] = 1.0
    consts_base[0, 640:1664] = bv

    maps = []
    for i in range(N_CORES):
        ci = slice(i * CSH, (i + 1) * CSH)
        # wp_p[p, k*CSH + c] = Wp[k*128 + p, ci_c]
        wp_p = np.ascontiguousarray(
            wp[:, ci].reshape(KC, 128, CSH).transpose(1, 0, 2).reshape(128, KC * CSH)
        ).astype(bf16)
        consts5 = consts_base.copy()
        consts5[4, 0:512] = np.tile(bp[ci], B)
        m = {
            "vfti": vfti,
            "wp_p": wp_p,
            "consts5": consts5.astype(bf16),
        }
        for k in range(KC):
            m[f"wv{k}"] = wv_chunks[k]
        maps.append(m)
    return maps


def run(inputs, trace=False, **kw):
    from concourse.bass_utils import run_bass_kernel_spmd

    nc = _get_built()
    res = run_bass_kernel_spmd(
        nc,
        make_in_maps(inputs),
        core_ids=list(range(N_CORES)),
        trace=trace,
        **kw,
    )
    full = np.empty((B, T, C), np.float32)
    for i, r in enumerate(res.results):
        # out[p, q, b, c] -> full[b, q*128 + p, ci_c]
        o = np.asarray(r["out"]).transpose(2, 1, 0, 3).reshape(B, T, CSH)
        full[:, :, i * CSH : (i + 1) * CSH] = o.astype(np.float32)
    return full, res


def kernel(**inputs) -> np.ndarray:
    full, _ = run(inputs, trace=False)
    return full
